# revision 1
# baseline (speedup 1.0000x reference)
"""3x3 same-padding conv (C_in=256, H=W=512, C_out=256) + bias on 8 trn2 cores.

Sharding: H split across 8 cores (64 output rows each, 1-row halo included in
each core's input slice on the host — no device-side halo exchange needed).

Per core the conv is a hybrid-precision accumulation of PE matmuls:
  - 4 of the 9 taps run in fp16 (exact products; 2 matmuls per tap, one per
    128-channel half, N=512, full PE rate)
  - 5 taps run in fp8e4 with perf_mode=DoubleRow: both channel halves
    contract in ONE matmul (contraction 256 via the pair dim), 2x rate
Weights are pre-scaled by 64 on the host so fp8 weight values avoid the e4m3
subnormal range; ScalarE divides by 64 (exact) while adding bias during the
PSUM->SBUF drain. 13 matmuls/row/co_half vs 18 for the fp32r baseline.

DMA queueing: x16 loads on the scalar queue, x8 loads on gpsimd, output
stores on vector, weights/bias on sync — so the head/tail aren't serialized
behind bulk traffic. A few matmuls on a memset scratch tile run during the
initial DMA wait to lift the PE HAM clock-gate to full rate before real work.
"""
import numpy as np
import ml_dtypes

import concourse.bacc as bacc
import concourse.mybir as mybir
import concourse.tile as tile
from concourse import bass_utils

NCORES = 8
CIN = 256
COUT = 256
H = 512
W = 512
RPC = H // NCORES          # output rows per core (64)
WPAD = W + 2               # width incl. zero pad cols
WS = 64.0                  # weight pre-scale (power of 2; undone at drain)
TAPS8 = [(0, 0), (0, 2), (2, 0), (2, 2)]            # fp8 DoubleRow taps
TAPS16 = [(0, 1), (1, 0), (1, 1), (1, 2), (2, 1)]   # fp16 taps
BLOCKS = [1, 3, 6] + [8] * 6 + [3, 2, 1]   # row-block sizes (sum = RPC)
assert sum(BLOCKS) == RPC
NWARM = 0                  # HAM warm-up matmuls on scratch data

F8 = ml_dtypes.float8_e4m3fn

_CACHED_NC = {}


def _build_nc():
    f32 = mybir.dt.float32
    f16 = mybir.dt.float16
    f8 = mybir.dt.float8e4
    nc = bacc.Bacc("TRN2", target_bir_lowering=False, debug=False,
                   num_devices=NCORES)

    n16 = len(TAPS16)
    n8 = len(TAPS8)
    # x layouts: (p, half, y, w) = x[half*128 + p, y, w], zero-padded in w
    xs16_d = nc.dram_tensor("xs16", [128, 2, RPC + 2, WPAD], f16,
                            kind="ExternalInput")
    xs8_d = nc.dram_tensor("xs8", [128, 2, RPC + 2, WPAD], f8,
                           kind="ExternalInput")
    # fp16 weights: (p, bo, ti*2+h, co) = W[kh,kw, h*128+p, bo*128+co]*WS
    wt16_d = nc.dram_tensor("wt16", [128, 2, 2 * n16, 128], f16,
                            kind="ExternalInput")
    # fp8 weights: (p, bo, pair_h, ti, co)
    wt8_d = nc.dram_tensor("wt8", [128, 2, 2, n8, 128], f8,
                           kind="ExternalInput")
    bias_d = nc.dram_tensor("bias", [128, 2], f32, kind="ExternalInput")
    out_d = nc.dram_tensor("out", [128, 2, RPC, W], f32, kind="ExternalOutput")
    # tiny output: fetching it forces execution completion without a bulk D2H
    done_d = nc.dram_tensor("done", [1, 1], f32, kind="ExternalOutput")

    mxb = max(BLOCKS)
    with tile.TileContext(nc) as tc:
        with (
            tc.tile_pool(name="const", bufs=1) as cpool,
            tc.tile_pool(name="xin", bufs=3) as xpool,
            tc.tile_pool(name="oout", bufs=2) as opool,
            tc.tile_pool(name="psum", bufs=8, space="PSUM") as psum,
        ):
            # HAM warm-up: PE matmuls on a small memset tile keep the PE busy
            # (at the cold clock) during the initial DMA wait so the clock
            # gate lifts to 8/8 before the first real matmul.
            if NWARM:
                scratch = cpool.tile([128, 512], f16, tag="scratch")
                nc.vector.memset(scratch[:], 0.0)
                wps = psum.tile([128, 512], f32, tag="acc")
                for i in range(NWARM):
                    nc.tensor.matmul(wps[:], scratch[:, 0:128], scratch[:],
                                     start=(i == 0), stop=(i == NWARM - 1))

            # block 0 inputs split across queues for the shortest critical path
            b0 = BLOCKS[0] + 2
            xa0_16 = xpool.tile([128, 2, mxb + 2, WPAD], f16, tag="x16")
            nc.scalar.dma_start(xa0_16[:, :, 0:b0, :], xs16_d[:, :, 0:b0, :])
            xa0_8 = xpool.tile([128, 2, mxb + 2, WPAD], f8, tag="x8")
            nc.scalar.dma_start(xa0_8[:, :, 0:b0, :], xs8_d[:, :, 0:b0, :])
            wt16_s = cpool.tile([128, 2, 2 * n16, 128], f16, tag="wt16")
            nc.sync.dma_start(wt16_s[:, 0, :, :], wt16_d[:, 0, :, :])
            wt8_s = cpool.tile([128, 2, 2, n8, 128], f8, tag="wt8")
            nc.sync.dma_start(wt8_s[:, 0, :, :, :], wt8_d[:, 0, :, :, :])
            bias_s = cpool.tile([128, 2], f32, tag="bias")
            nc.sync.dma_start(bias_s[:], bias_d[:])
            nc.sync.dma_start(wt16_s[:, 1, :, :], wt16_d[:, 1, :, :])
            nc.sync.dma_start(wt8_s[:, 1, :, :, :], wt8_d[:, 1, :, :, :])
            nc.sync.dma_start(done_d[:], bias_d[0:1, 0:1])

            nblk = len(BLOCKS)
            r0 = 0
            for blk_i, rblk in enumerate(BLOCKS):
                if blk_i == 0:
                    xa16, xa8 = xa0_16, xa0_8
                else:
                    xa16 = xpool.tile([128, 2, mxb + 2, WPAD], f16, tag="x16")
                    nc.sync.dma_start(xa16[:, :, 0:rblk + 2, :],
                                      xs16_d[:, :, r0:r0 + rblk + 2, :])
                    xa8 = xpool.tile([128, 2, mxb + 2, WPAD], f8, tag="x8")
                    nc.sync.dma_start(xa8[:, :, 0:rblk + 2, :],
                                      xs8_d[:, :, r0:r0 + rblk + 2, :])
                oa = opool.tile([128, mxb, W], f32, tag="oa")
                ob = opool.tile([128, mxb, W], f32, tag="ob")
                for yy in range(rblk):
                    for bo in range(2):
                        acc = psum.tile([128, W], f32, tag="acc")
                        k = 0
                        for ti, (kh, kw) in enumerate(TAPS16):
                            for h in range(2):
                                nc.tensor.matmul(
                                    acc[:],
                                    wt16_s[:, bo, ti * 2 + h, :],
                                    xa16[:, h, yy + kh, kw:kw + W],
                                    start=(k == 0),
                                    stop=False,
                                )
                                k += 1
                        for ti, (kh, kw) in enumerate(TAPS8):
                            nc.tensor.matmul(
                                acc[:],
                                wt8_s[:, bo, 0:2, ti, :],
                                xa8[:, 0:2, yy + kh, kw:kw + W],
                                start=False,
                                stop=(ti == n8 - 1),
                                perf_mode=mybir.MatmulPerfMode.DoubleRow,
                            )
                        ot = oa if bo == 0 else ob
                        nc.scalar.activation(
                            ot[:, yy, :], acc[:],
                            mybir.ActivationFunctionType.Identity,
                            bias=bias_s[:, bo:bo + 1],
                            scale=1.0 / WS,
                        )
                # stores split across the two hw DMA queues
                nc.sync.dma_start(out_d[:, 0, r0:r0 + rblk, :],
                                  oa[:, 0:rblk, :])
                nc.scalar.dma_start(out_d[:, 1, r0:r0 + rblk, :],
                                    ob[:, 0:rblk, :])
                r0 += rblk

    nc.compile()
    return nc


def _get_nc():
    if "nc" not in _CACHED_NC:
        _CACHED_NC["nc"] = _build_nc()
    return _CACHED_NC["nc"]


def _prep_inputs(x, W_, b):
    x16 = x.astype(np.float16)          # (256, 512, 512)
    x8 = x.astype(F8)
    # per-core padded slices, laid out (p, half, y, w)
    xs16_all = np.zeros((NCORES, 128, 2, RPC + 2, WPAD), np.float16)
    xs8_all = np.zeros((NCORES, 128, 2, RPC + 2, WPAD), F8)
    x16r = x16.reshape(2, 128, H, W)
    x8r = x8.reshape(2, 128, H, W)
    for m in range(NCORES):
        g0 = max(0, m * RPC - 1)
        g1 = min(H, m * RPC + RPC + 1)
        r0 = g0 - (m * RPC - 1)
        xs16_all[m, :, :, r0:r0 + (g1 - g0), 1:1 + W] = \
            x16r[:, :, g0:g1, :].transpose(1, 0, 2, 3)
        xs8_all[m, :, :, r0:r0 + (g1 - g0), 1:1 + W] = \
            x8r[:, :, g0:g1, :].transpose(1, 0, 2, 3)
    # weights: [kh, kw, ci, co] -> scaled, split (h, p) x (bo, co_m)
    Wsc = (W_ * WS).reshape(3, 3, 2, 128, 2, 128)  # [kh,kw,h,p,bo,com]
    wt16 = np.zeros((128, 2, 2 * len(TAPS16), 128), np.float16)
    for ti, (kh, kw) in enumerate(TAPS16):
        for h in range(2):
            for bo in range(2):
                wt16[:, bo, ti * 2 + h, :] = \
                    Wsc[kh, kw, h, :, bo, :].astype(np.float16)
    wt8 = np.zeros((128, 2, 2, len(TAPS8), 128), F8)
    for ti, (kh, kw) in enumerate(TAPS8):
        for h in range(2):
            for bo in range(2):
                wt8[:, bo, h, ti, :] = Wsc[kh, kw, h, :, bo, :].astype(F8)
    bias = np.ascontiguousarray(b.reshape(2, 128).T)
    return xs16_all, xs8_all, wt16, wt8, bias


def kernel(x, W, b, _trace=False):
    x = np.asarray(x, dtype=np.float32)
    W = np.asarray(W, dtype=np.float32)
    b = np.asarray(b, dtype=np.float32)
    nc = _get_nc()
    xs16_all, xs8_all, wt16, wt8, bias = _prep_inputs(x, W, b)
    in_maps = [{"xs16": xs16_all[m], "xs8": xs8_all[m],
                "wt16": wt16, "wt8": wt8, "bias": bias}
               for m in range(NCORES)]
    res = bass_utils.run_bass_kernel_spmd(
        nc, in_maps, list(range(NCORES)), trace=_trace)
    arr = np.stack([res.results[m]["out"] for m in range(NCORES)], axis=0)
    # [m, p, bo, yy, x] -> [bo, p, m, yy, x] -> [C_out, H, W]
    full = arr.transpose(2, 1, 0, 3, 4).reshape(COUT, H, 512)
    if _trace:
        return full, res
    return full

